# revision 3
# baseline (speedup 1.0000x reference)
"""MinLSTM cell kernel for 8x Trainium2 NeuronCores — transfer-optimized.

The graded number is the wall-clock of a warm kernel(**inputs) call. With
axon-tunneled devices that wall time is dominated by host-side work, not
device compute (~1ms):

  - run_bass_kernel_spmd re-creates a fresh jax.jit closure per call (full
    retrace + XLA/walrus recompile every time)  -> cache ONE jit here.
  - the tunnel moves ~80MB/s h2d, ~60MB/s d2h   -> ship x as f16 (67MB) and
    h back as rounded uint8 (33.5MB). f16 is the input floor: the forget
    gate random-walks input quant noise over T=512 steps, so int8 x blows
    up worst-case error (measured 2.7e-1); uint8 h costs only a half-step
    3.9e-3 against the 2e-2 tolerance.
  - per-call numpy transposes of 134MB are seconds on this 1-core host
    -> x goes up in natural [B,T,D] layout (device DVE stream-transposes
    it), h comes back already in [B,T,U] layout (device PE-transposes),
    so the host does one cheap astype each way and zero reshuffles.

Device-side design (data-parallel over batch, 32 rows/core):
  - x chunk [32(b), TC*D] f16 is DVE stream-transposed (32x32 blocks) into
    x^T tiles [128(d), (t,b)] feeding the fused input GEMM
    xw = x @ [Wf|Wi|Wc] (f16 weights, psum f32, per-partition bias folded
    into the eviction, layout [128(n), (t, gate)] f32).
  - scan state s = sigma(2c) (so h = 2s-1 = tanh(c)); per step an identity
    matmul preloads xw into PSUM off the critical path, 12 fp32r matmuls
    (U2 stationary) accumulate the recurrent term, ScalarE applies
    sigma/tanh straight out of PSUM, DVE does c = f*c + i*cc.
  - h = 2s-1 (f16, ScalarE) is PE-transposed to [(j,b), u] and staged to
    [64, TC*128] as uint8 round(127h)+128, then two plain DMAs per chunk
    write hout[b, t, u] with contiguous runs. The global concat over cores
    IS [B,T,U]; the host dequantizes with one 256-entry LUT gather.
"""
import os
# The axon NTFF profile hook module is absent in this container; a stray
# BASS_TRACE=1 in the environment would crash the PJRT exec path.
os.environ["BASS_NEVER_TRACE"] = "1"

import numpy as np
from contextlib import ExitStack

import jax
import concourse.bass as bass
import concourse.bacc as bacc
import concourse.tile as tile
import concourse.mybir as mybir
from concourse import bass2jax

F32 = mybir.dt.float32
F32R = mybir.dt.float32r
F16 = mybir.dt.float16
BF16 = mybir.dt.bfloat16
I8 = mybir.dt.int8
U8 = mybir.dt.uint8
AF = mybir.ActivationFunctionType
OP = mybir.AluOpType

B, T, D, U3, UN = 256, 512, 256, 768, 256
NCORES = 8
BC = B // NCORES          # 32 batch rows per core
TC = 32                   # timesteps per chunk


def _build(t_total=T):
    nchunk = t_total // TC
    nc = bacc.Bacc("TRN2", target_bir_lowering=False, debug=False)

    xin = nc.declare_dram_parameter("xin", [BC, t_total, D], F16, isOutput=False)
    wt = nc.declare_dram_parameter("wt", [D, U3], F16, isOutput=False)
    uh = nc.declare_dram_parameter("uh", [D, U3], F32R, isOutput=False)
    bp = nc.declare_dram_parameter("bp", [128, 6], F32, isOutput=False)
    idp = nc.declare_dram_parameter("idp", [128, 128], F32R, isOutput=False)
    idh = nc.declare_dram_parameter("idh", [128, 128], F16, isOutput=False)
    s0 = nc.declare_dram_parameter("s0", [128, 64], F32R, isOutput=False)
    c0 = nc.declare_dram_parameter("c0", [128, 64], F32, isOutput=False)
    hout = nc.declare_dram_parameter("hout", [BC, t_total, UN], U8, isOutput=True)
    s_out = nc.declare_dram_parameter("s_out", [128, 64], F32R, isOutput=True)
    c_out = nc.declare_dram_parameter("c_out", [128, 64], F32, isOutput=True)

    with tile.TileContext(nc) as tc, ExitStack() as ctx:
        const = ctx.enter_context(tc.tile_pool(name="const", bufs=1))
        xn_pool = ctx.enter_context(tc.tile_pool(name="xn", bufs=2))
        xt_pool = ctx.enter_context(tc.tile_pool(name="xt", bufs=2))
        xw_pool = ctx.enter_context(tc.tile_pool(name="xw", bufs=2))
        ho_pool = ctx.enter_context(tc.tile_pool(name="ho", bufs=2))
        work = ctx.enter_context(tc.tile_pool(name="work", bufs=3))
        ps_g = ctx.enter_context(tc.tile_pool(name="psg", bufs=2, space="PSUM"))
        ps_s = ctx.enter_context(tc.tile_pool(name="pss", bufs=2, space="PSUM"))
        ps_t = ctx.enter_context(tc.tile_pool(name="pst", bufs=2, space="PSUM"))

        # constants / persistent state
        w_sb = const.tile([128, 2 * U3], F16)        # W tiles: [:, 768k + n]
        uh_sb = const.tile([128, 2 * U3], F32R)      # 2*U tiles, same packing
        bp_sb = const.tile([128, 6], F32)
        idp_sb = const.tile([128, 128], F32R)
        idh_sb = const.tile([128, 128], F16)
        s_sb = const.tile([128, 64], F32R)           # sigma(2c), col = 32j + b
        c_sb = const.tile([128, 64], F32)
        for k in range(2):
            nc.sync.dma_start(w_sb[:, k * U3:(k + 1) * U3], wt[k * 128:(k + 1) * 128, :])
            nc.sync.dma_start(uh_sb[:, k * U3:(k + 1) * U3], uh[k * 128:(k + 1) * 128, :])
        nc.sync.dma_start(bp_sb[:], bp[:])
        nc.sync.dma_start(idp_sb[:], idp[:])
        nc.sync.dma_start(idh_sb[:], idh[:])
        nc.sync.dma_start(s_sb[:], s0[:])
        nc.sync.dma_start(c_sb[:], c0[:])

        for ch in range(nchunk):
            t0 = ch * TC
            # ---- load natural x chunk [32(b), (t', d)]: 16KB/partition ----
            xn_sb = xn_pool.tile([BC, TC * D], F16, tag="xn")
            nc.sync.dma_start(xn_sb[:], xin[:, t0:t0 + TC, :])
            xn_v = xn_sb[:].rearrange("p (t d) -> p t d", d=D)

            # ---- DVE stream-transpose to x^T tiles [128(d), (t', b)] ----
            xt_t0 = xt_pool.tile([128, TC * BC], F16, tag="xt0")
            xt_t1 = xt_pool.tile([128, TC * BC], F16, tag="xt1")
            xt_k = (xt_t0, xt_t1)
            xo_v = [t[:].rearrange("p (t b) -> p t b", b=BC) for t in xt_k]
            for tp in range(TC):
                for k in range(2):
                    for s4 in range(4):
                        dlo = 128 * k + 32 * s4
                        nc.vector.transpose(
                            xo_v[k][32 * s4:32 * s4 + 32, tp, :],
                            xn_v[:, tp, dlo:dlo + 32],
                        )

            # ---- xw GEMM for this chunk: out[n-tile jj, (t', gate)] ----
            xw_sb = xw_pool.tile([128, TC * 192], F32R, tag="xw")
            xw_v = xw_sb[:].rearrange("p (t g) -> p t g", g=192)
            nhalves = (TC * BC) // 512
            for jj in range(6):
                for nh in range(nhalves):
                    psg = ps_g.tile([128, 512], F32, tag="psg")
                    for k in range(2):
                        nc.tensor.matmul(
                            psg[:],
                            w_sb[:, k * U3 + 128 * jj: k * U3 + 128 * jj + 128],
                            xt_k[k][:, nh * 512:(nh + 1) * 512],
                            start=(k == 0), stop=(k == 1),
                        )
                    # evict + per-partition bias add
                    nc.vector.tensor_scalar(
                        xw_v[:, nh * 16:(nh + 1) * 16, 32 * jj:32 * jj + 32],
                        psg[:].rearrange("p (t g) -> p t g", g=32),
                        bp_sb[:, jj:jj + 1], None, op0=OP.add,
                    )

            # ---- output staging: [(j,b), (t', u_sub)] f16 ----
            ho_sb = ho_pool.tile([64, TC * 128], U8, tag="ho")

            # ---- the sequential scan ----
            for tp in range(TC):
                psfi = ps_s.tile([128, 128], F32, tag="psfi")
                pscc = ps_s.tile([128, 64], F32, tag="pscc")
                nc.tensor.matmul(psfi[:], idp_sb[:], xw_v[:, tp, 0:128],
                                 start=True, stop=False, skip_group_check=True)
                nc.tensor.matmul(pscc[:], idp_sb[:], xw_v[:, tp, 128:192],
                                 start=True, stop=False, skip_group_check=True)
                for jj in range(4):
                    for k in range(2):
                        nc.tensor.matmul(
                            psfi[:, 32 * jj:32 * jj + 32],
                            uh_sb[:, k * U3 + 128 * jj: k * U3 + 128 * jj + 128],
                            s_sb[:, 32 * k:32 * k + 32],
                            start=False, stop=(jj == 3 and k == 1),
                            skip_group_check=True,
                        )
                fi = work.tile([128, 128], F32, tag="fi")
                nc.scalar.activation(fi[:], psfi[:], AF.Sigmoid)
                for jj in range(4, 6):
                    for k in range(2):
                        nc.tensor.matmul(
                            pscc[:, 32 * (jj - 4):32 * (jj - 4) + 32],
                            uh_sb[:, k * U3 + 128 * jj: k * U3 + 128 * jj + 128],
                            s_sb[:, 32 * k:32 * k + 32],
                            start=False, stop=(jj == 5 and k == 1),
                            skip_group_check=True,
                        )
                cc = work.tile([128, 64], F32, tag="cc")
                nc.scalar.activation(cc[:], pscc[:], AF.Tanh)
                m1 = work.tile([128, 64], F32, tag="m1")
                nc.vector.tensor_tensor(m1[:], fi[:, 0:64], c_sb[:], op=OP.mult)
                m2 = work.tile([128, 64], F32, tag="m2")
                nc.vector.tensor_tensor(m2[:], fi[:, 64:128], cc[:], op=OP.mult)
                nc.vector.tensor_tensor(c_sb[:], m1[:], m2[:], op=OP.add)
                nc.scalar.activation(s_sb[:], c_sb[:], AF.Sigmoid, scale=2.0)
                # h = 2*s - 1 (= tanh(c)), f16, off the s-recurrence chain
                ht = work.tile([128, 64], F16, tag="ht")
                nc.scalar.activation(ht[:], s_sb[:].bitcast(F32), AF.Copy,
                                     bias=-1.0, scale=2.0)
                # PE-transpose h to [(j,b), u_sub] and stage as f16
                pst = ps_t.tile([64, 128], F16, tag="pst")
                nc.tensor.transpose(pst[:], ht[:], idh_sb[:])
                # q = convert(127h + 128) in u8; HW converts round-to-nearest
                # (CoreSim truncates -- sim shows a half-step worse, HW is truth)
                nc.vector.tensor_scalar(
                    ho_sb[:, tp * 128:(tp + 1) * 128], pst[:],
                    127.0, 128.0, op0=OP.mult, op1=OP.add)

            # two plain DMAs: rows 0:32 are j=0 (u 0:128), rows 32:64 j=1
            ho_v = ho_sb[:].rearrange("p (t u) -> p t u", u=128)
            nc.sync.dma_start(hout[:, t0:t0 + TC, 0:128], ho_v[0:32, :, :])
            nc.sync.dma_start(hout[:, t0:t0 + TC, 128:256], ho_v[32:64, :, :])

        # final recurrent state, so calls can be chained over time segments
        nc.sync.dma_start(s_out[:], s_sb[:])
        nc.sync.dma_start(c_out[:], c_sb[:])

    nc.compile()
    return nc


# ---------------------------------------------------------------------------
# host runner: one cached jit over shard_map(bass_exec), reused every call
# ---------------------------------------------------------------------------

_RUNNER = None
TSEG = 512            # timesteps per device call (T -> single call, fastest)


def _make_runner():
    nc = _build(t_total=TSEG)
    bass2jax.install_neuronx_cc_hook()

    partition_name = (nc.partition_id_tensor.name
                      if nc.partition_id_tensor else None)
    in_names, out_names, out_avals = [], [], []
    for alloc in nc.m.functions[0].allocations:
        if not isinstance(alloc, mybir.MemoryLocationSet):
            continue
        name = alloc.memorylocations[0].name
        if alloc.kind == "ExternalInput":
            if name != partition_name:
                in_names.append(name)
        elif alloc.kind == "ExternalOutput":
            out_names.append(name)
            out_avals.append(jax.core.ShapedArray(
                tuple(alloc.tensor_shape), mybir.dt.np(alloc.dtype)))
    bind_names = list(in_names)
    if partition_name is not None:
        bind_names.append(partition_name)

    def _body(*args):
        operands = list(args)
        if partition_name is not None:
            operands.append(bass2jax.partition_id_tensor())
        outs = bass2jax._bass_exec_p.bind(
            *operands,
            out_avals=tuple(out_avals),
            in_names=tuple(bind_names),
            out_names=tuple(out_names),
            lowering_input_output_aliases=(),
            sim_require_finite=True,
            sim_require_nnan=True,
            nc=nc,
        )
        return tuple(outs)

    from jax.sharding import Mesh, PartitionSpec
    from jax.experimental.shard_map import shard_map

    devices = jax.devices()[:NCORES]
    assert len(devices) == NCORES
    mesh = Mesh(np.asarray(devices), ("core",))
    sharded = {"xin": True, "s0": True, "c0": True, "hout": True}
    in_specs = tuple(
        PartitionSpec("core") if sharded.get(n) else PartitionSpec()
        for n in in_names)
    out_specs = tuple(PartitionSpec("core") for _ in out_names)
    fn = jax.jit(shard_map(
        _body, mesh=mesh, in_specs=in_specs, out_specs=out_specs,
        check_rep=False))
    return fn, in_names, out_names


def kernel(x, Wf, Uf, bf, Wi, Ui, bi, Wc, Uc, bc, h0, c0):
    global _RUNNER
    if _RUNNER is None:
        _RUNNER = _make_runner()
    fn, in_names, out_names = _RUNNER

    W = np.concatenate([np.asarray(Wf), np.asarray(Wi), np.asarray(Wc)],
                       axis=1).astype(np.float32)
    Ucat = np.concatenate([np.asarray(Uf), np.asarray(Ui), np.asarray(Uc)],
                          axis=1).astype(np.float32)
    bcat = np.concatenate([np.asarray(bf), np.asarray(bi), np.asarray(bc)]
                          ).astype(np.float32)
    h0 = np.asarray(h0, dtype=np.float32)
    c0 = np.asarray(c0, dtype=np.float32)

    Uh2 = 2.0 * Ucat                                   # s @ (2U), s = (h+1)/2
    bias = bcat - Ucat.sum(axis=0)                     # absorbs the -1 of 2s-1
    bp2 = np.empty((128, 6), np.float32)
    for jj in range(6):
        bp2[:, jj] = bias[128 * jj:128 * (jj + 1)]

    # per-core state tiles [128, 64] with col = 32j + b, partition = u % 128
    s0g = np.empty((NCORES * 128, 64), np.float32)
    c0g = np.empty((NCORES * 128, 64), np.float32)
    for r in range(NCORES):
        h0s = h0[r * BC:(r + 1) * BC]
        c0s = c0[r * BC:(r + 1) * BC]
        for j in range(2):
            s0g[r * 128:(r + 1) * 128, 32 * j:32 * (j + 1)] = \
                (h0s[:, 128 * j:128 * (j + 1)].T + 1.0) / 2.0
            c0g[r * 128:(r + 1) * 128, 32 * j:32 * (j + 1)] = \
                c0s[:, 128 * j:128 * (j + 1)].T

    arrs = {
        "wt": W.astype(np.float16),
        "uh": Uh2,
        "bp": bp2,
        "idp": np.eye(128, dtype=np.float32),
        "idh": np.eye(128, dtype=np.float16),
        "s0": s0g,
        "c0": c0g,
    }
    x = np.asarray(x)

    global _WARMED
    if not _WARMED:
        # first call pays compile anyway; run the steady-state path once more
        # so subsequent (timed) calls skip first-warm dispatch/allocator costs
        _run(fn, in_names, out_names, dict(arrs), x)
        _WARMED = True
    return _run(fn, in_names, out_names, arrs, x)


_WARMED = False


def _run(fn, in_names, out_names, arrs, x):
    oidx = {n: i for i, n in enumerate(out_names)}
    # time-segmented device calls chained through the on-device (s, c) state.
    # With TSEG == T this is one call; smaller TSEG pipelines host casts and
    # dequants against wire time (measured: the per-call overhead eats the
    # overlap gain on this tunnel, so TSEG = T is the default).
    results = []
    for t0 in range(0, T, TSEG):
        arrs["xin"] = np.asarray(x[:, t0:t0 + TSEG], dtype=np.float16)
        r = fn(*[arrs[n] for n in in_names])
        arrs["s0"] = r[oidx["s_out"]]
        arrs["c0"] = r[oidx["c_out"]]
        results.append(r[oidx["hout"]])

    lut = ((np.arange(256) - 128.0) / 127.0).astype(np.float32)
    out = np.empty((B, T, UN), np.float32)
    for i, r in enumerate(results):
        out[:, i * TSEG:(i + 1) * TSEG] = lut[np.asarray(r)]
    return out


# revision 4
# speedup vs baseline: 1.3583x; 1.3583x over previous
"""MinLSTM cell kernel for 8x Trainium2 NeuronCores — transfer-optimized.

The graded number is the wall-clock of a warm kernel(**inputs) call. With
axon-tunneled devices that wall time is dominated by host-side work, not
device compute (~1ms):

  - run_bass_kernel_spmd re-creates a fresh jax.jit closure per call (full
    retrace + XLA/walrus recompile every time)  -> cache ONE jit here.
  - the tunnel moves ~80MB/s h2d, ~60MB/s d2h   -> ship x as f16 (67MB) and
    h back as rounded uint8 (33.5MB). f16 is the input floor: the forget
    gate random-walks input quant noise over T=512 steps, so int8 x blows
    up worst-case error (measured 2.7e-1); uint8 h costs only a half-step
    3.9e-3 against the 2e-2 tolerance.
  - per-call numpy transposes of 134MB are seconds on this 1-core host
    -> x goes up in natural [B,T,D] layout (device DVE stream-transposes
    it), h comes back already in [B,T,U] layout (device PE-transposes),
    so the host does one cheap astype each way and zero reshuffles.

Device-side design (data-parallel over batch, 32 rows/core):
  - x chunk [32(b), TC*D] f16 is DVE stream-transposed (32x32 blocks) into
    x^T tiles [128(d), (t,b)] feeding the fused input GEMM
    xw = x @ [Wf|Wi|Wc] (f16 weights, psum f32, per-partition bias folded
    into the eviction, layout [128(n), (t, gate)] f32).
  - scan state s = sigma(2c) (so h = 2s-1 = tanh(c)); per step an identity
    matmul preloads xw into PSUM off the critical path, 12 fp32r matmuls
    (U2 stationary) accumulate the recurrent term, ScalarE applies
    sigma/tanh straight out of PSUM, DVE does c = f*c + i*cc.
  - h = 2s-1 (f16, ScalarE) is PE-transposed to [(j,b), u] and staged to
    [64, TC*128] as uint8 round(127h)+128, then two plain DMAs per chunk
    write hout[b, t, u] with contiguous runs. The global concat over cores
    IS [B,T,U]; the host dequantizes with one 256-entry LUT gather.
"""
import os
# The axon NTFF profile hook module is absent in this container; a stray
# BASS_TRACE=1 in the environment would crash the PJRT exec path.
os.environ["BASS_NEVER_TRACE"] = "1"

import numpy as np
from contextlib import ExitStack

import jax
import concourse.bass as bass
import concourse.bacc as bacc
import concourse.tile as tile
import concourse.mybir as mybir
from concourse import bass2jax

F32 = mybir.dt.float32
F32R = mybir.dt.float32r
F16 = mybir.dt.float16
BF16 = mybir.dt.bfloat16
I8 = mybir.dt.int8
U8 = mybir.dt.uint8
AF = mybir.ActivationFunctionType
OP = mybir.AluOpType

B, T, D, U3, UN = 256, 512, 256, 768, 256
NCORES = 8
BC = B // NCORES          # 32 batch rows per core
TC = 32                   # timesteps per chunk


def _build(t_total=T):
    nchunk = t_total // TC
    nc = bacc.Bacc("TRN2", target_bir_lowering=False, debug=False)

    xin = nc.declare_dram_parameter("xin", [BC, t_total, D], F16, isOutput=False)
    wt = nc.declare_dram_parameter("wt", [D, U3], F16, isOutput=False)
    uh = nc.declare_dram_parameter("uh", [D, U3], F32R, isOutput=False)
    bp = nc.declare_dram_parameter("bp", [128, 6], F32, isOutput=False)
    idp = nc.declare_dram_parameter("idp", [128, 128], F32R, isOutput=False)
    idh = nc.declare_dram_parameter("idh", [128, 128], F16, isOutput=False)
    s0 = nc.declare_dram_parameter("s0", [128, 64], F32R, isOutput=False)
    c0 = nc.declare_dram_parameter("c0", [128, 64], F32, isOutput=False)
    hout = nc.declare_dram_parameter("hout", [BC, t_total, UN], U8, isOutput=True)
    s_out = nc.declare_dram_parameter("s_out", [128, 64], F32R, isOutput=True)
    c_out = nc.declare_dram_parameter("c_out", [128, 64], F32, isOutput=True)

    with tile.TileContext(nc) as tc, ExitStack() as ctx:
        const = ctx.enter_context(tc.tile_pool(name="const", bufs=1))
        xn_pool = ctx.enter_context(tc.tile_pool(name="xn", bufs=2))
        xt_pool = ctx.enter_context(tc.tile_pool(name="xt", bufs=2))
        xw_pool = ctx.enter_context(tc.tile_pool(name="xw", bufs=2))
        ho_pool = ctx.enter_context(tc.tile_pool(name="ho", bufs=2))
        work = ctx.enter_context(tc.tile_pool(name="work", bufs=3))
        ps_g = ctx.enter_context(tc.tile_pool(name="psg", bufs=2, space="PSUM"))
        ps_s = ctx.enter_context(tc.tile_pool(name="pss", bufs=2, space="PSUM"))
        ps_t = ctx.enter_context(tc.tile_pool(name="pst", bufs=2, space="PSUM"))

        # constants / persistent state
        w_sb = const.tile([128, 2 * U3], F16)        # W tiles: [:, 768k + n]
        uh_sb = const.tile([128, 2 * U3], F32R)      # 2*U tiles, same packing
        bp_sb = const.tile([128, 6], F32)
        idp_sb = const.tile([128, 128], F32R)
        idh_sb = const.tile([128, 128], F16)
        s_sb = const.tile([128, 64], F32R)           # sigma(2c), col = 32j + b
        c_sb = const.tile([128, 64], F32)
        for k in range(2):
            nc.sync.dma_start(w_sb[:, k * U3:(k + 1) * U3], wt[k * 128:(k + 1) * 128, :])
            nc.sync.dma_start(uh_sb[:, k * U3:(k + 1) * U3], uh[k * 128:(k + 1) * 128, :])
        nc.sync.dma_start(bp_sb[:], bp[:])
        nc.sync.dma_start(idp_sb[:], idp[:])
        nc.sync.dma_start(idh_sb[:], idh[:])
        nc.sync.dma_start(s_sb[:], s0[:])
        nc.sync.dma_start(c_sb[:], c0[:])

        for ch in range(nchunk):
            t0 = ch * TC
            # ---- load natural x chunk [32(b), (t', d)]: 16KB/partition ----
            xn_sb = xn_pool.tile([BC, TC * D], F16, tag="xn")
            nc.sync.dma_start(xn_sb[:], xin[:, t0:t0 + TC, :])
            xn_v = xn_sb[:].rearrange("p (t d) -> p t d", d=D)

            # ---- DVE stream-transpose to x^T tiles [128(d), (t', b)] ----
            xt_t0 = xt_pool.tile([128, TC * BC], F16, tag="xt0")
            xt_t1 = xt_pool.tile([128, TC * BC], F16, tag="xt1")
            xt_k = (xt_t0, xt_t1)
            xo_v = [t[:].rearrange("p (t b) -> p t b", b=BC) for t in xt_k]
            for tp in range(TC):
                for k in range(2):
                    for s4 in range(4):
                        dlo = 128 * k + 32 * s4
                        nc.vector.transpose(
                            xo_v[k][32 * s4:32 * s4 + 32, tp, :],
                            xn_v[:, tp, dlo:dlo + 32],
                        )

            # ---- xw GEMM for this chunk: out[n-tile jj, (t', gate)] ----
            xw_sb = xw_pool.tile([128, TC * 192], F32R, tag="xw")
            xw_v = xw_sb[:].rearrange("p (t g) -> p t g", g=192)
            nhalves = (TC * BC) // 512
            for jj in range(6):
                for nh in range(nhalves):
                    psg = ps_g.tile([128, 512], F32, tag="psg")
                    for k in range(2):
                        nc.tensor.matmul(
                            psg[:],
                            w_sb[:, k * U3 + 128 * jj: k * U3 + 128 * jj + 128],
                            xt_k[k][:, nh * 512:(nh + 1) * 512],
                            start=(k == 0), stop=(k == 1),
                        )
                    # evict + per-partition bias add
                    nc.vector.tensor_scalar(
                        xw_v[:, nh * 16:(nh + 1) * 16, 32 * jj:32 * jj + 32],
                        psg[:].rearrange("p (t g) -> p t g", g=32),
                        bp_sb[:, jj:jj + 1], None, op0=OP.add,
                    )

            # ---- output staging: [(j,b), (t', u_sub)] f16 ----
            ho_sb = ho_pool.tile([64, TC * 128], U8, tag="ho")

            # ---- the sequential scan ----
            for tp in range(TC):
                psfi = ps_s.tile([128, 128], F32, tag="psfi")
                pscc = ps_s.tile([128, 64], F32, tag="pscc")
                nc.tensor.matmul(psfi[:], idp_sb[:], xw_v[:, tp, 0:128],
                                 start=True, stop=False, skip_group_check=True)
                nc.tensor.matmul(pscc[:], idp_sb[:], xw_v[:, tp, 128:192],
                                 start=True, stop=False, skip_group_check=True)
                for jj in range(4):
                    for k in range(2):
                        nc.tensor.matmul(
                            psfi[:, 32 * jj:32 * jj + 32],
                            uh_sb[:, k * U3 + 128 * jj: k * U3 + 128 * jj + 128],
                            s_sb[:, 32 * k:32 * k + 32],
                            start=False, stop=(jj == 3 and k == 1),
                            skip_group_check=True,
                        )
                fi = work.tile([128, 128], F32, tag="fi")
                nc.scalar.activation(fi[:], psfi[:], AF.Sigmoid)
                for jj in range(4, 6):
                    for k in range(2):
                        nc.tensor.matmul(
                            pscc[:, 32 * (jj - 4):32 * (jj - 4) + 32],
                            uh_sb[:, k * U3 + 128 * jj: k * U3 + 128 * jj + 128],
                            s_sb[:, 32 * k:32 * k + 32],
                            start=False, stop=(jj == 5 and k == 1),
                            skip_group_check=True,
                        )
                cc = work.tile([128, 64], F32, tag="cc")
                nc.scalar.activation(cc[:], pscc[:], AF.Tanh)
                m1 = work.tile([128, 64], F32, tag="m1")
                nc.vector.tensor_tensor(m1[:], fi[:, 0:64], c_sb[:], op=OP.mult)
                m2 = work.tile([128, 64], F32, tag="m2")
                nc.vector.tensor_tensor(m2[:], fi[:, 64:128], cc[:], op=OP.mult)
                nc.vector.tensor_tensor(c_sb[:], m1[:], m2[:], op=OP.add)
                nc.scalar.activation(s_sb[:], c_sb[:], AF.Sigmoid, scale=2.0)
                # h = 2*s - 1 (= tanh(c)), f16, off the s-recurrence chain
                ht = work.tile([128, 64], F16, tag="ht")
                nc.scalar.activation(ht[:], s_sb[:].bitcast(F32), AF.Copy,
                                     bias=-1.0, scale=2.0)
                # PE-transpose h to [(j,b), u_sub] and stage as f16
                pst = ps_t.tile([64, 128], F16, tag="pst")
                nc.tensor.transpose(pst[:], ht[:], idh_sb[:])
                # q = convert(127h + 128) in u8; HW converts round-to-nearest
                # (CoreSim truncates -- sim shows a half-step worse, HW is truth)
                nc.vector.tensor_scalar(
                    ho_sb[:, tp * 128:(tp + 1) * 128], pst[:],
                    127.0, 128.0, op0=OP.mult, op1=OP.add)

            # two plain DMAs: rows 0:32 are j=0 (u 0:128), rows 32:64 j=1
            ho_v = ho_sb[:].rearrange("p (t u) -> p t u", u=128)
            nc.sync.dma_start(hout[:, t0:t0 + TC, 0:128], ho_v[0:32, :, :])
            nc.sync.dma_start(hout[:, t0:t0 + TC, 128:256], ho_v[32:64, :, :])

        # final recurrent state, so calls can be chained over time segments
        nc.sync.dma_start(s_out[:], s_sb[:])
        nc.sync.dma_start(c_out[:], c_sb[:])

    nc.compile()
    return nc


# ---------------------------------------------------------------------------
# host runner: one cached jit over shard_map(bass_exec), reused every call
# ---------------------------------------------------------------------------

_RUNNER = None
TSEG = 512            # timesteps per device call (T -> single call, fastest)


def _make_runner():
    nc = _build(t_total=TSEG)
    bass2jax.install_neuronx_cc_hook()

    partition_name = (nc.partition_id_tensor.name
                      if nc.partition_id_tensor else None)
    in_names, out_names, out_avals = [], [], []
    for alloc in nc.m.functions[0].allocations:
        if not isinstance(alloc, mybir.MemoryLocationSet):
            continue
        name = alloc.memorylocations[0].name
        if alloc.kind == "ExternalInput":
            if name != partition_name:
                in_names.append(name)
        elif alloc.kind == "ExternalOutput":
            out_names.append(name)
            out_avals.append(jax.core.ShapedArray(
                tuple(alloc.tensor_shape), mybir.dt.np(alloc.dtype)))
    bind_names = list(in_names)
    if partition_name is not None:
        bind_names.append(partition_name)

    def _body(*args):
        operands = list(args)
        if partition_name is not None:
            operands.append(bass2jax.partition_id_tensor())
        outs = bass2jax._bass_exec_p.bind(
            *operands,
            out_avals=tuple(out_avals),
            in_names=tuple(bind_names),
            out_names=tuple(out_names),
            lowering_input_output_aliases=(),
            sim_require_finite=True,
            sim_require_nnan=True,
            nc=nc,
        )
        return tuple(outs)

    from jax.sharding import Mesh, PartitionSpec
    from jax.experimental.shard_map import shard_map

    devices = jax.devices()[:NCORES]
    assert len(devices) == NCORES
    mesh = Mesh(np.asarray(devices), ("core",))
    sharded = {"xin": True, "s0": True, "c0": True, "hout": True}
    in_specs = tuple(
        PartitionSpec("core") if sharded.get(n) else PartitionSpec()
        for n in in_names)
    out_specs = tuple(PartitionSpec("core") for _ in out_names)
    fn = jax.jit(shard_map(
        _body, mesh=mesh, in_specs=in_specs, out_specs=out_specs,
        check_rep=False))
    return fn, in_names, out_names


def kernel(x, Wf, Uf, bf, Wi, Ui, bi, Wc, Uc, bc, h0, c0):
    global _RUNNER
    if _RUNNER is None:
        _RUNNER = _make_runner()
    fn, in_names, out_names = _RUNNER

    W = np.concatenate([np.asarray(Wf), np.asarray(Wi), np.asarray(Wc)],
                       axis=1).astype(np.float32)
    Ucat = np.concatenate([np.asarray(Uf), np.asarray(Ui), np.asarray(Uc)],
                          axis=1).astype(np.float32)
    bcat = np.concatenate([np.asarray(bf), np.asarray(bi), np.asarray(bc)]
                          ).astype(np.float32)
    h0 = np.asarray(h0, dtype=np.float32)
    c0 = np.asarray(c0, dtype=np.float32)

    Uh2 = 2.0 * Ucat                                   # s @ (2U), s = (h+1)/2
    bias = bcat - Ucat.sum(axis=0)                     # absorbs the -1 of 2s-1
    bp2 = np.empty((128, 6), np.float32)
    for jj in range(6):
        bp2[:, jj] = bias[128 * jj:128 * (jj + 1)]

    # per-core state tiles [128, 64] with col = 32j + b, partition = u % 128
    s0g = np.empty((NCORES * 128, 64), np.float32)
    c0g = np.empty((NCORES * 128, 64), np.float32)
    for r in range(NCORES):
        h0s = h0[r * BC:(r + 1) * BC]
        c0s = c0[r * BC:(r + 1) * BC]
        for j in range(2):
            s0g[r * 128:(r + 1) * 128, 32 * j:32 * (j + 1)] = \
                (h0s[:, 128 * j:128 * (j + 1)].T + 1.0) / 2.0
            c0g[r * 128:(r + 1) * 128, 32 * j:32 * (j + 1)] = \
                c0s[:, 128 * j:128 * (j + 1)].T

    arrs = {
        "wt": W.astype(np.float16),
        "uh": Uh2,
        "bp": bp2,
        "idp": np.eye(128, dtype=np.float32),
        "idh": np.eye(128, dtype=np.float16),
        "s0": s0g,
        "c0": c0g,
    }
    x = np.asarray(x)

    global _WARMED
    if not _WARMED:
        # first call pays compile anyway; run the steady-state path once more
        # so subsequent (timed) calls skip first-warm dispatch/allocator costs
        _run(fn, in_names, out_names, dict(arrs), x)
        _WARMED = True
    return _run(fn, in_names, out_names, arrs, x)


_WARMED = False
_XBUF = {}        # per-segment f16 staging buffers, internal-only, reused
                  # across calls (safe: jax copies args before kernel returns)


def _run(fn, in_names, out_names, arrs, x):
    oidx = {n: i for i, n in enumerate(out_names)}
    # time-segmented device calls chained through the on-device (s, c) state.
    # With TSEG == T this is one call; smaller TSEG pipelines host casts and
    # dequants against wire time (measured: the per-call overhead eats the
    # overlap gain on this tunnel, so TSEG = T is the default).
    results = []
    for t0 in range(0, T, TSEG):
        buf = _XBUF.get(t0)
        if buf is None:
            buf = _XBUF[t0] = np.empty((B, TSEG, D), np.float16)
        np.copyto(buf, x[:, t0:t0 + TSEG], casting="unsafe")
        arrs["xin"] = buf
        r = fn(*[arrs[n] for n in in_names])
        arrs["s0"] = r[oidx["s_out"]]
        arrs["c0"] = r[oidx["c_out"]]
        results.append(r[oidx["hout"]])

    # dequant q -> (q-128)/127 in two in-place passes, no temps
    out = np.empty((B, T, UN), np.float32)
    for i, r in enumerate(results):
        sl = out[:, i * TSEG:(i + 1) * TSEG]
        np.subtract(np.asarray(r), np.float32(128.0), dtype=np.float32, out=sl)
        np.multiply(sl, np.float32(1.0 / 127.0), out=sl)
    return out
